# revision 26
# baseline (speedup 1.0000x reference)
"""BitLinear (activation int8-quant + ternary weight) + squared-ReLU on 8 Trainium2
NeuronCores.

Sharding: tensor-parallel over weight rows (out_features). Each core receives the
full activation tensor and a 1/8 slice of the weight matrix, computes its slice of
the GEMM + squared ReLU, and the host concatenates the slices.

The global weight scale mean(|W|) is computed on-device: per-core partial sums +
a scalar AllReduce across the 8 cores. A small head of activation tiles is
quantized + transposed in the collective's shadow so the matmul stream starts
with a ready backlog.

Math notes:
  - x_q = round(x * 127/scale) with scale = clip(amax_row(|x|), 1e-5). Values are
    integers in [-127, 127] -> exactly representable in bf16 (8 significand bits).
  - w_q in {-1, 0, 1} = (w > 0.5*ws) - (w < -0.5*ws) via exact fp32 strict
    compares.
  - The bf16 GEMM with fp32 PSUM accumulation is exact (all products are small
    integers, partial sums < 2^24).
  - x rounding reproduces fp32 round-to-nearest-even (jnp.round semantics) with
    the +1.5*2^23 magic-constant trick, applied after the product is rounded to
    fp32 (same double-rounding as the reference).
  - squared ReLU with the dequant scale folded in:
    out = Square(Relu((w_scale/scale) * psum)).
"""

import sys

if "/opt/trn_rl_repo" not in sys.path:
    sys.path.insert(0, "/opt/trn_rl_repo")

import numpy as np

import concourse.bacc as bacc
import concourse.bass_isa as bass_isa
import concourse.mybir as mybir
import concourse.tile as tile
from concourse.bass_utils import run_bass_kernel_spmd
from concourse.masks import make_identity
from concourse.tile import add_dep_helper

dt = mybir.dt
NCORES = 8
C_MAGIC = 1.5 * 2**23  # fp32 round-to-nearest-even forcing constant
HEAD = 6               # x tiles pre-processed in the collective's shadow

# Stash of the most recent BassKernelResults (test harness reads exec_time_ns).
LAST_RESULTS = None

_NC_CACHE = {}


def _build(T, K, O, max_val):
    """Build + compile the per-core Bass module.

    Per-core tensors: x [T, K] f32 (replicated), w [O, K] f32 (this core's rows),
    out [T, O] f32.
    """
    assert T % 128 == 0 and K % 256 == 0 and O % 512 == 0
    TT = T // 128     # token tiles
    KT = K // 128     # contraction tiles
    OC = O // 512     # psum-width output chunks per core
    OT = O // 128     # weight row tiles
    n_w_elem = float(NCORES * O * K)

    nc = bacc.Bacc("TRN2", target_bir_lowering=False, debug=False,
                   num_devices=NCORES)

    x_ap = nc.dram_tensor("x", [T, K], dt.float32, kind="ExternalInput").ap()
    w_ap = nc.dram_tensor("w", [O, K], dt.float32, kind="ExternalInput").ap()
    out_ap = nc.dram_tensor("out", [T, O], dt.float32, kind="ExternalOutput").ap()

    with tile.TileContext(nc) as tc:
        with (
            tc.tile_pool(name="const", bufs=1) as const_pool,
            tc.tile_pool(name="wres", bufs=1) as wres_pool,
            tc.tile_pool(name="xs", bufs=3) as x_pool,
            tc.tile_pool(name="xq", bufs=2) as xq_pool,
            tc.tile_pool(name="xqt", bufs=12) as xqt_pool,
            tc.tile_pool(name="osb", bufs=2) as out_pool,
            tc.tile_pool(name="sc", bufs=8) as sc_pool,
            tc.tile_pool(name="tps", bufs=3, space="PSUM") as tps_pool,
            tc.tile_pool(name="mmps", bufs=2, space="PSUM") as mm_pool,
            tc.tile_pool(name="dram", bufs=2, space="DRAM") as dram_pool,
        ):
            ident = const_pool.tile([128, 128], dt.bfloat16)
            make_identity(nc, ident[:])

            wqT_cs = [wres_pool.tile([128, KT * 512], dt.bfloat16,
                                     name=f"wqT{c}") for c in range(OC)]
            ws = wres_pool.tile([128, 1], dt.float32)          # w_scale

            def x_quant(t, head=False):
                # DMA + per-token scales + exact quantization + PE transposes
                # for token tile t; returns (xqT, g). Only the tiny g op
                # depends on the collective result ws.
                xt = x_pool.tile([128, K], dt.float32, tag="x", name="x")
                nc.sync.dma_start(xt[:], x_ap[128 * t:128 * (t + 1), :])

                amax = sc_pool.tile([128, 1], dt.float32, tag="amax",
                                    name="amax")
                nc.vector.tensor_reduce(amax[:], xt[:],
                                        axis=mybir.AxisListType.X,
                                        op=mybir.AluOpType.max,
                                        apply_absolute_value=True)
                nc.vector.tensor_scalar_max(amax[:], amax[:], 1e-5)
                rinv = sc_pool.tile([128, 1], dt.float32, tag="rinv",
                                    name="rinv")
                nc.vector.reciprocal(rinv[:], amax[:])
                rs = sc_pool.tile([128, 1], dt.float32, tag="rs", name="rs")
                nc.vector.tensor_scalar_mul(rs[:], rinv[:], float(max_val))
                g = sc_pool.tile([128, 1], dt.float32, tag="g", name="g")
                nc.vector.tensor_tensor(g[:], ws[:], rinv[:],
                                        op=mybir.AluOpType.mult)

                # x_q = rint(fl(x * rs)), processed in independent k-halves
                # so transposes/evacs/matmuls pipeline at half-tile grain
                # (Tile dependencies are whole-tile). Head tiles evacuate on
                # ACT only, keeping the DVE free for the startup ramp.
                half = KT // 2
                HK = half * 128
                xqTs = []
                for hh in range(2):
                    xqf = xq_pool.tile([128, HK], dt.float32, tag=f"xqf{hh}",
                                       name="xqf")
                    nc.scalar.activation(xqf[:], xt[:, HK * hh:HK * (hh + 1)],
                                         mybir.ActivationFunctionType.Copy,
                                         scale=rs[:])
                    xq = xq_pool.tile([128, HK], dt.bfloat16, tag=f"xq{hh}",
                                      name="xq")
                    nc.vector.tensor_scalar(xq[:], xqf[:], C_MAGIC, C_MAGIC,
                                            op0=mybir.AluOpType.add,
                                            op1=mybir.AluOpType.subtract)

                    xqT = xqt_pool.tile([128, HK], dt.bfloat16, tag=f"xqT{hh}",
                                        name="xqT")
                    ps = tps_pool.tile([128, HK], dt.bfloat16,
                                       tag="tps", name="ps")
                    for j in range(half):
                        nc.tensor.transpose(
                            ps[:, 128 * j:128 * (j + 1)],
                            xq[:, 128 * j:128 * (j + 1)], ident[:])
                    if head or hh == 0:
                        nc.scalar.copy(xqT[:], ps[:])
                    else:
                        nc.vector.tensor_copy(xqT[:], ps[:])
                    xqTs.append(xqT)
                return xqTs, g

            # ------------- weight phase (staging pools freed after) -------------
            with (
                tc.tile_pool(name="wstage", bufs=2) as wstage_pool,
                tc.tile_pool(name="wq", bufs=2) as wq_pool,
            ):
                # pass 1: stream w tiles, two-stage |w| partial sums
                wpart = wres_pool.tile([128, OT * KT], dt.float32)
                for r in range(OT):
                    wt = wstage_pool.tile([128, K], dt.float32, tag="wstage")
                    nc.sync.dma_start(wt[:], w_ap[128 * r:128 * (r + 1), :])
                    nc.vector.tensor_reduce(
                        wpart[:, KT * r:KT * (r + 1)],
                        wt[:].rearrange("p (a b) -> p a b", b=128),
                        axis=mybir.AxisListType.X,
                        op=mybir.AluOpType.add, apply_absolute_value=True)
                wpart1 = wres_pool.tile([128, 1], dt.float32)
                nc.vector.tensor_reduce(wpart1[:], wpart[:],
                                        axis=mybir.AxisListType.X,
                                        op=mybir.AluOpType.add)
                wtot = wres_pool.tile([128, 1], dt.float32)
                nc.gpsimd.partition_all_reduce(wtot[:], wpart1[:], channels=128,
                                               reduce_op=bass_isa.ReduceOp.add)
                # scalar AllReduce across the 8 cores via DRAM bounce buffers
                cc_in = dram_pool.tile([1, 1], dt.float32)
                cc_out = dram_pool.tile([1, 1], dt.float32)
                nc.gpsimd.dma_start(cc_in[:], wtot[0:1, 0:1])
                nc.gpsimd.collective_compute(
                    "AllReduce", mybir.AluOpType.add,
                    replica_groups=[list(range(NCORES))],
                    ins=[cc_in.opt()], outs=[cc_out.opt()])
                wsum_bc = wres_pool.tile([128, 1], dt.float32)
                nc.gpsimd.dma_start(wsum_bc[:], cc_out[:].broadcast_to([128, 1]))

                nc.vector.tensor_scalar_mul(ws[:], wsum_bc[:], 1.0 / n_w_elem)
                halfws = wres_pool.tile([128, 1], dt.float32)  # +0.5 * w_scale
                nc.vector.tensor_scalar_mul(halfws[:], ws[:], 0.5)
                neghws = wres_pool.tile([128, 1], dt.float32)  # -0.5 * w_scale
                nc.vector.tensor_scalar_mul(neghws[:], ws[:], -0.5)

                # head of x tiles, processed in the collective's shadow (only
                # each head tile's tiny g op actually waits for ws)
                head_tiles = [x_quant(t, head=True) for t in range(HEAD)]

                # pass 2: re-stream w; w_q = (w > 0.5ws) - (w < -0.5ws) via
                # exact strict compares; per-r PE transposes pipeline along
                wq_tt_insts = []
                for r in range(OT):
                    wt = wstage_pool.tile([128, K], dt.float32, tag="wstage")
                    nc.sync.dma_start(wt[:], w_ap[128 * r:128 * (r + 1), :])
                    tp = wstage_pool.tile([128, K], dt.float32, tag="wquant",
                                          bufs=1)
                    nc.vector.tensor_scalar(tp[:], wt[:], halfws[:], None,
                                            op0=mybir.AluOpType.is_gt)
                    tn = wstage_pool.tile([128, K], dt.float32, tag="wquant2",
                                          bufs=1)
                    nc.vector.tensor_scalar(tn[:], wt[:], neghws[:], None,
                                            op0=mybir.AluOpType.is_lt)
                    wq = wq_pool.tile([128, K], dt.bfloat16, tag="wq")
                    wq_tt = nc.vector.tensor_tensor(wq[:], tp[:], tn[:],
                                                    op=mybir.AluOpType.subtract)
                    wq_tt_insts.append(wq_tt)
                    c, rr = r // 4, r % 4
                    wqT3 = wqT_cs[c][:].rearrange("p (j o) -> p j o", o=512)
                    for b in range(KT // 4):
                        ps = tps_pool.tile([128, 512], dt.bfloat16, tag="tps",
                                           name="psw")
                        for q in range(4):
                            j = 4 * b + q
                            nc.tensor.transpose(
                                ps[:, 128 * q:128 * (q + 1)],
                                wq[:, 128 * j:128 * (j + 1)], ident[:])
                        dst = wqT3[:, 4 * b:4 * b + 4, 128 * rr:128 * (rr + 1)]
                        psv = ps[:].rearrange("p (q t) -> p q t", t=128)
                        if b % 2 == 0:
                            nc.scalar.copy(dst, psv)
                        else:
                            nc.vector.tensor_copy(dst, psv)

            # HAM warmup: back-to-back identity matmuls so the PE clock is at
            # 8/8 when the real stream starts; gated on the first wq
            warm_ps = mm_pool.tile([128, 128], dt.float32, tag="warm", bufs=1)
            for i in range(64):
                mm = nc.tensor.matmul(warm_ps[:], ident[:], ident[:],
                                      start=True, stop=True)
                if i == 0:
                    add_dep_helper(mm.ins, wq_tt_insts[0].ins, sync=True,
                                   reason="HAM warmup during weight phase tail")

            # ---------------- main loop over token tiles ----------------
            for t in range(TT):
                if t < HEAD:
                    xqTs, g = head_tiles[t]
                else:
                    xqTs, g = x_quant(t)

                # GEMM: psum[t, o] += xqT[k, t].T @ wqT[k, o]
                psums = [mm_pool.tile([128, 512], dt.float32, tag=f"mm{c}",
                                      name=f"mm{c}")
                         for c in range(OC)]
                half = KT // 2
                for c in range(OC):
                    for j in range(KT):
                        xh = xqTs[j // half]
                        lhsT = xh[:, 128 * (j % half):128 * (j % half + 1)]
                        nc.tensor.matmul(
                            psums[c][:], lhsT,
                            wqT_cs[c][:, 512 * j:512 * (j + 1)],
                            start=(j == 0), stop=(j == KT - 1))

                # out = Square(Relu(g * psum))
                osb = out_pool.tile([128, O], dt.float32, tag="osb", name="osb")
                for c in range(OC):
                    nc.scalar.activation(osb[:, 512 * c:512 * (c + 1)],
                                         psums[c][:],
                                         mybir.ActivationFunctionType.Relu,
                                         scale=g[:])
                sq = out_pool.tile([128, O], dt.float32, tag="sq", name="sq")
                nc.scalar.activation(sq[:], osb[:],
                                     mybir.ActivationFunctionType.Square)
                nc.sync.dma_start(out_ap[128 * t:128 * (t + 1), :], sq[:])

    nc.compile()
    return nc


def _get_nc(T, K, O, max_val):
    key = (T, K, O, max_val)
    if key not in _NC_CACHE:
        _NC_CACHE[key] = _build(T, K, O, max_val)
    return _NC_CACHE[key]


def kernel(x, weight, bits=8):
    global LAST_RESULTS
    x = np.asarray(x, dtype=np.float32)
    weight = np.asarray(weight, dtype=np.float32)
    bits = int(bits)
    max_val = (1 << (bits - 1)) - 1

    lead_shape = x.shape[:-1]
    K = x.shape[-1]
    T = int(np.prod(lead_shape))
    O_total, K_w = weight.shape
    assert K == K_w and O_total % NCORES == 0
    O = O_total // NCORES

    nc = _get_nc(T, K, O, max_val)

    x2 = np.ascontiguousarray(x.reshape(T, K))
    in_maps = [{"x": x2, "w": np.ascontiguousarray(weight[i * O:(i + 1) * O])}
               for i in range(NCORES)]
    res = run_bass_kernel_spmd(nc, in_maps, list(range(NCORES)))
    LAST_RESULTS = res

    out = np.concatenate([res.results[i]["out"] for i in range(NCORES)], axis=1)
    return out.reshape(*lead_shape, O_total)


# revision 28
# speedup vs baseline: 1.0001x; 1.0001x over previous
"""BitLinear (activation int8-quant + ternary weight) + squared-ReLU on 8 Trainium2
NeuronCores.

Sharding: tensor-parallel over weight rows (out_features). Each core receives the
full activation tensor and a 1/8 slice of the weight matrix, computes its slice of
the GEMM + squared ReLU, and the host concatenates the slices.

The global weight scale mean(|W|) is computed on-device: per-core partial sums +
a scalar AllReduce across the 8 cores. A small head of activation tiles is
quantized + transposed in the collective's shadow so the matmul stream starts
with a ready backlog.

Math notes:
  - x_q = round(x * 127/scale) with scale = clip(amax_row(|x|), 1e-5). Values are
    integers in [-127, 127] -> exactly representable in bf16 (8 significand bits).
  - w_q in {-1, 0, 1} = (w > 0.5*ws) - (w < -0.5*ws) via exact fp32 strict
    compares.
  - The bf16 GEMM with fp32 PSUM accumulation is exact (all products are small
    integers, partial sums < 2^24).
  - x rounding reproduces fp32 round-to-nearest-even (jnp.round semantics) with
    the +1.5*2^23 magic-constant trick, applied after the product is rounded to
    fp32 (same double-rounding as the reference).
  - squared ReLU with the dequant scale folded in:
    out = Square(Relu((w_scale/scale) * psum)).
"""

import sys

if "/opt/trn_rl_repo" not in sys.path:
    sys.path.insert(0, "/opt/trn_rl_repo")

import numpy as np

import concourse.bacc as bacc
import concourse.bass_isa as bass_isa
import concourse.mybir as mybir
import concourse.tile as tile
from concourse.bass_utils import run_bass_kernel_spmd
from concourse.masks import make_identity
from concourse.tile import add_dep_helper

dt = mybir.dt
NCORES = 8
C_MAGIC = 1.5 * 2**23  # fp32 round-to-nearest-even forcing constant
HEAD = 6               # x tiles pre-processed in the collective's shadow

# Stash of the most recent BassKernelResults (test harness reads exec_time_ns).
LAST_RESULTS = None

_NC_CACHE = {}


def _build(T, K, O, max_val):
    """Build + compile the per-core Bass module.

    Per-core tensors: x [T, K] f32 (replicated), w [O, K] f32 (this core's rows),
    out [T, O] f32.
    """
    assert T % 128 == 0 and K % 256 == 0 and O % 512 == 0
    TT = T // 128     # token tiles
    KT = K // 128     # contraction tiles
    OC = O // 512     # psum-width output chunks per core
    OT = O // 128     # weight row tiles
    n_w_elem = float(NCORES * O * K)

    nc = bacc.Bacc("TRN2", target_bir_lowering=False, debug=False,
                   num_devices=NCORES)

    x_ap = nc.dram_tensor("x", [T, K], dt.float32, kind="ExternalInput").ap()
    w_ap = nc.dram_tensor("w", [O, K], dt.float32, kind="ExternalInput").ap()
    out_ap = nc.dram_tensor("out", [T, O], dt.float32, kind="ExternalOutput").ap()

    with tile.TileContext(nc) as tc:
        with (
            tc.tile_pool(name="const", bufs=1) as const_pool,
            tc.tile_pool(name="wres", bufs=1) as wres_pool,
            tc.tile_pool(name="xs", bufs=3) as x_pool,
            tc.tile_pool(name="xq", bufs=2) as xq_pool,
            tc.tile_pool(name="xqt", bufs=12) as xqt_pool,
            tc.tile_pool(name="osb", bufs=2) as out_pool,
            tc.tile_pool(name="sc", bufs=8) as sc_pool,
            tc.tile_pool(name="tps", bufs=3, space="PSUM") as tps_pool,
            tc.tile_pool(name="mmps", bufs=2, space="PSUM") as mm_pool,
            tc.tile_pool(name="dram", bufs=2, space="DRAM") as dram_pool,
        ):
            ident = const_pool.tile([128, 128], dt.bfloat16)
            make_identity(nc, ident[:])

            wqT_cs = [wres_pool.tile([128, KT * 512], dt.bfloat16,
                                     name=f"wqT{c}") for c in range(OC)]
            ws = wres_pool.tile([128, 1], dt.float32)          # w_scale

            def x_quant(t):
                # DMA + per-token scales + exact quantization + PE transposes
                # for token tile t; returns (xqT, g). Only the tiny g op
                # depends on the collective result ws.
                xt = x_pool.tile([128, K], dt.float32, tag="x", name="x")
                nc.sync.dma_start(xt[:], x_ap[128 * t:128 * (t + 1), :])

                amax = sc_pool.tile([128, 1], dt.float32, tag="amax",
                                    name="amax")
                nc.vector.tensor_reduce(amax[:], xt[:],
                                        axis=mybir.AxisListType.X,
                                        op=mybir.AluOpType.max,
                                        apply_absolute_value=True)
                nc.vector.tensor_scalar_max(amax[:], amax[:], 1e-5)
                rinv = sc_pool.tile([128, 1], dt.float32, tag="rinv",
                                    name="rinv")
                nc.vector.reciprocal(rinv[:], amax[:])
                rs = sc_pool.tile([128, 1], dt.float32, tag="rs", name="rs")
                nc.vector.tensor_scalar_mul(rs[:], rinv[:], float(max_val))
                g = sc_pool.tile([128, 1], dt.float32, tag="g", name="g")
                nc.vector.tensor_tensor(g[:], ws[:], rinv[:],
                                        op=mybir.AluOpType.mult)

                # x_q = rint(fl(x * rs)): fp32 product on ACT, then RNE to
                # integer via +C/-C on DVE, cast to exact bf16 integers
                xqf = xq_pool.tile([128, K], dt.float32, tag="xqf", name="xqf")
                nc.scalar.activation(xqf[:], xt[:],
                                     mybir.ActivationFunctionType.Copy,
                                     scale=rs[:])
                xq = xq_pool.tile([128, K], dt.bfloat16, tag="xq", name="xq")
                nc.vector.tensor_scalar(xq[:], xqf[:], C_MAGIC, C_MAGIC,
                                        op0=mybir.AluOpType.add,
                                        op1=mybir.AluOpType.subtract)

                # transpose xq -> xqT [128, KT*128] bf16 (k on partitions)
                xqT = xqt_pool.tile([128, KT * 128], dt.bfloat16, tag="xqT",
                                    name="xqT")
                half = KT // 2
                for hh in range(2):
                    ps = tps_pool.tile([128, half * 128], dt.bfloat16,
                                       tag="tps", name="ps")
                    for j in range(half):
                        jj = hh * half + j
                        nc.tensor.transpose(
                            ps[:, 128 * j:128 * (j + 1)],
                            xq[:, 128 * jj:128 * (jj + 1)], ident[:])
                    dst = xqT[:, 128 * half * hh:128 * half * (hh + 1)]
                    if hh == 0:
                        nc.scalar.copy(dst, ps[:])
                    else:
                        nc.vector.tensor_copy(dst, ps[:])
                return xqT, g

            # ------------- weight phase (staging pools freed after) -------------
            with (
                tc.tile_pool(name="wstage", bufs=2) as wstage_pool,
                tc.tile_pool(name="wq", bufs=2) as wq_pool,
            ):
                # pass 1: stream w tiles; |w| per-partition sums on ACT
                # (accum_out), keeping the DVE free for the activation head
                wpart = wres_pool.tile([128, OT], dt.float32)
                for r in range(OT):
                    wt = wstage_pool.tile([128, K], dt.float32, tag="wstage")
                    nc.sync.dma_start(wt[:], w_ap[128 * r:128 * (r + 1), :])
                    wabs = wstage_pool.tile([128, K], dt.float32, tag="wabs")
                    nc.scalar.activation(wabs[:], wt[:],
                                         mybir.ActivationFunctionType.Abs,
                                         accum_out=wpart[:, r:r + 1])
                wpart1 = wres_pool.tile([128, 1], dt.float32)
                nc.vector.tensor_reduce(wpart1[:], wpart[:],
                                        axis=mybir.AxisListType.X,
                                        op=mybir.AluOpType.add)
                wtot = wres_pool.tile([128, 1], dt.float32)
                nc.gpsimd.partition_all_reduce(wtot[:], wpart1[:], channels=128,
                                               reduce_op=bass_isa.ReduceOp.add)
                # scalar AllReduce across the 8 cores via DRAM bounce buffers
                cc_in = dram_pool.tile([1, 1], dt.float32)
                cc_out = dram_pool.tile([1, 1], dt.float32)
                nc.gpsimd.dma_start(cc_in[:], wtot[0:1, 0:1])
                nc.gpsimd.collective_compute(
                    "AllReduce", mybir.AluOpType.add,
                    replica_groups=[list(range(NCORES))],
                    ins=[cc_in.opt()], outs=[cc_out.opt()])
                wsum_bc = wres_pool.tile([128, 1], dt.float32)
                nc.gpsimd.dma_start(wsum_bc[:], cc_out[:].broadcast_to([128, 1]))

                nc.vector.tensor_scalar_mul(ws[:], wsum_bc[:], 1.0 / n_w_elem)
                halfws = wres_pool.tile([128, 1], dt.float32)  # +0.5 * w_scale
                nc.vector.tensor_scalar_mul(halfws[:], ws[:], 0.5)
                neghws = wres_pool.tile([128, 1], dt.float32)  # -0.5 * w_scale
                nc.vector.tensor_scalar_mul(neghws[:], ws[:], -0.5)

                # head of x tiles, processed in the collective's shadow (only
                # each head tile's tiny g op actually waits for ws)
                head_tiles = [x_quant(t) for t in range(HEAD)]

                # pass 2: re-stream w; w_q = (w > 0.5ws) - (w < -0.5ws) via
                # exact strict compares; per-r PE transposes pipeline along
                wq_tt_insts = []
                for r in range(OT):
                    wt = wstage_pool.tile([128, K], dt.float32, tag="wstage")
                    nc.sync.dma_start(wt[:], w_ap[128 * r:128 * (r + 1), :])
                    tp = wstage_pool.tile([128, K], dt.float32, tag="wquant",
                                          bufs=1)
                    nc.vector.tensor_scalar(tp[:], wt[:], halfws[:], None,
                                            op0=mybir.AluOpType.is_gt)
                    tn = wstage_pool.tile([128, K], dt.float32, tag="wquant2",
                                          bufs=1)
                    nc.vector.tensor_scalar(tn[:], wt[:], neghws[:], None,
                                            op0=mybir.AluOpType.is_lt)
                    wq = wq_pool.tile([128, K], dt.bfloat16, tag="wq")
                    wq_tt = nc.vector.tensor_tensor(wq[:], tp[:], tn[:],
                                                    op=mybir.AluOpType.subtract)
                    wq_tt_insts.append(wq_tt)
                    c, rr = r // 4, r % 4
                    wqT3 = wqT_cs[c][:].rearrange("p (j o) -> p j o", o=512)
                    for b in range(KT // 4):
                        ps = tps_pool.tile([128, 512], dt.bfloat16, tag="tps",
                                           name="psw")
                        for q in range(4):
                            j = 4 * b + q
                            nc.tensor.transpose(
                                ps[:, 128 * q:128 * (q + 1)],
                                wq[:, 128 * j:128 * (j + 1)], ident[:])
                        dst = wqT3[:, 4 * b:4 * b + 4, 128 * rr:128 * (rr + 1)]
                        psv = ps[:].rearrange("p (q t) -> p q t", t=128)
                        if b % 2 == 0:
                            nc.scalar.copy(dst, psv)
                        else:
                            nc.vector.tensor_copy(dst, psv)

            # HAM warmup: back-to-back identity matmuls so the PE clock is at
            # 8/8 when the real stream starts; gated on the first wq
            warm_ps = mm_pool.tile([128, 128], dt.float32, tag="warm", bufs=1)
            for i in range(64):
                mm = nc.tensor.matmul(warm_ps[:], ident[:], ident[:],
                                      start=True, stop=True)
                if i == 0:
                    add_dep_helper(mm.ins, wq_tt_insts[0].ins, sync=True,
                                   reason="HAM warmup during weight phase tail")

            # ---------------- main loop over token tiles ----------------
            for t in range(TT):
                if t < HEAD:
                    xqT, g = head_tiles[t]
                else:
                    xqT, g = x_quant(t)

                # GEMM: psum[t, o] += xqT[k, t].T @ wqT[k, o]
                psums = [mm_pool.tile([128, 512], dt.float32, tag=f"mm{c}",
                                      name=f"mm{c}")
                         for c in range(OC)]
                for c in range(OC):
                    for j in range(KT):
                        lhsT = xqT[:, 128 * j:128 * (j + 1)]
                        nc.tensor.matmul(
                            psums[c][:], lhsT,
                            wqT_cs[c][:, 512 * j:512 * (j + 1)],
                            start=(j == 0), stop=(j == KT - 1))

                # out = Square(Relu(g * psum))
                osb = out_pool.tile([128, O], dt.float32, tag="osb", name="osb")
                for c in range(OC):
                    nc.scalar.activation(osb[:, 512 * c:512 * (c + 1)],
                                         psums[c][:],
                                         mybir.ActivationFunctionType.Relu,
                                         scale=g[:])
                sq = out_pool.tile([128, O], dt.float32, tag="sq", name="sq")
                nc.scalar.activation(sq[:], osb[:],
                                     mybir.ActivationFunctionType.Square)
                nc.sync.dma_start(out_ap[128 * t:128 * (t + 1), :], sq[:])

    nc.compile()
    return nc


def _get_nc(T, K, O, max_val):
    key = (T, K, O, max_val)
    if key not in _NC_CACHE:
        _NC_CACHE[key] = _build(T, K, O, max_val)
    return _NC_CACHE[key]


def kernel(x, weight, bits=8):
    global LAST_RESULTS
    x = np.asarray(x, dtype=np.float32)
    weight = np.asarray(weight, dtype=np.float32)
    bits = int(bits)
    max_val = (1 << (bits - 1)) - 1

    lead_shape = x.shape[:-1]
    K = x.shape[-1]
    T = int(np.prod(lead_shape))
    O_total, K_w = weight.shape
    assert K == K_w and O_total % NCORES == 0
    O = O_total // NCORES

    nc = _get_nc(T, K, O, max_val)

    x2 = np.ascontiguousarray(x.reshape(T, K))
    in_maps = [{"x": x2, "w": np.ascontiguousarray(weight[i * O:(i + 1) * O])}
               for i in range(NCORES)]
    res = run_bass_kernel_spmd(nc, in_maps, list(range(NCORES)))
    LAST_RESULTS = res

    out = np.concatenate([res.results[i]["out"] for i in range(NCORES)], axis=1)
    return out.reshape(*lead_shape, O_total)
